# revision 9
# baseline (speedup 1.0000x reference)
"""Trainium2 Bass kernel for nn_MixNet (Mixing-method coordinate descent).

Sharding: data-parallel over batch B=64 across 8 NeuronCores (8 batch
elements per core); C is replicated. Each core runs 10 sweeps of block
coordinate descent over the 1024 coordinates in reference order
(8 blocks of 128), where the sequential within-block Gauss-Seidel update
is reproduced by M_INNER fixed-point iterations of

    x^m = normalize(A + L_blk @ (x^{m-1} - x^0))

which converges geometrically (~0.3x error per iteration) to the exact
sequential result. All heavy work is tensor-engine matmuls plus a
[128 x 256]-wide vectorized normalize per iteration.

Device layout (per core):
  V-hat [1024, 256]: free entries of V (coordinate-major; 256 = 8 batch x 32 K),
      frozen (is_input) entries are zero; their contribution to the matvec is a
      constant A_frozen = (-4C) @ V_frozen, computed once and re-seeded into
      PSUM each block via an identity matmul.
  All matmuls carry a factor of -4 folded into C so that the per-row scale is
  exactly rs = Dsqrt(0.25 * sum_k G^2) = 1/||G|| with the normalize sign
  (v = -g/||g||) absorbed, and a +1e30 mask column appended to the squared
  tile makes frozen rows come out ~1e-15 (i.e. zero) without a select op.
"""

import numpy as np

N_IN, AUX = 768, 255
NV = N_IN + 1 + AUX          # 1024 variables
KK = 32                      # embedding dim K
B = 64                       # full batch
NCORES = 8
BL = B // NCORES             # batch per core = 8
NBK = BL * KK                # 256 free width per core
P = 128                      # partitions / block size
NB = NV // P                 # 8 coordinate blocks
SWEEPS = 10
M_INNER = 3                  # fixed-point iterations per block
BIGMASK = 1e30
RSQRT_ONE_OP = False         # Abs_reciprocal_sqrt is inaccurate on HW; use Sqrt+reciprocal

_CACHE = {}


def _build(M=M_INNER, sweeps=SWEEPS):
    from contextlib import ExitStack

    import concourse.bass as bass
    import concourse.mybir as mybir
    import concourse.tile as tile
    from concourse import bacc

    f32 = mybir.dt.float32
    nc = bacc.Bacc(
        "TRN2", target_bir_lowering=False, debug=False, enable_asserts=False
    )
    cn4 = nc.dram_tensor("cn4", [NV, NV], f32, kind="ExternalInput").ap()
    u4 = nc.dram_tensor("u4", [NB, P, P], f32, kind="ExternalInput").ap()
    u4n = nc.dram_tensor("u4n", [NB, P, P], f32, kind="ExternalInput").ap()
    vh0 = nc.dram_tensor("vh0", [NV, NBK], f32, kind="ExternalInput").ap()
    vfz = nc.dram_tensor("vfz", [NV, NBK], f32, kind="ExternalInput").ap()
    msk = nc.dram_tensor("msk", [P, NB, BL], f32, kind="ExternalInput").ap()
    idn = nc.dram_tensor("idn", [P, P], f32, kind="ExternalInput").ap()
    vout = nc.dram_tensor("vout", [NV, NBK], f32, kind="ExternalOutput").ap()
    vout_t = vout.rearrange("(jc p) n -> p jc n", p=P)

    with tile.TileContext(nc) as tc, ExitStack() as ctx:
        const = ctx.enter_context(tc.tile_pool(name="const", bufs=1))
        stat = ctx.enter_context(tc.tile_pool(name="stat", bufs=4))
        xbp = ctx.enter_context(tc.tile_pool(name="xb", bufs=3))
        psum = ctx.enter_context(tc.tile_pool(name="psum", bufs=2, space="PSUM"))

        CN = const.tile([P, NB, NV], f32)
        U4 = const.tile([P, NB, P], f32)
        U4N = const.tile([P, NB, P], f32)
        VH = const.tile([P, NB, NBK], f32)
        VFZ = const.tile([P, NB, NBK], f32)
        AF = const.tile([P, NB, NBK], f32)
        SQ = const.tile([P, NB, BL, KK + 1], f32)
        ID = const.tile([P, P], f32)

        nc.sync.dma_start(out=CN, in_=cn4.rearrange("(jc p) i -> p jc i", p=P))
        nc.sync.dma_start(out=U4, in_=u4.rearrange("b j i -> j b i"))
        nc.sync.dma_start(out=U4N, in_=u4n.rearrange("b j i -> j b i"))
        nc.sync.dma_start(out=VH, in_=vh0.rearrange("(jc p) n -> p jc n", p=P))
        nc.sync.dma_start(out=VFZ, in_=vfz.rearrange("(jc p) n -> p jc n", p=P))
        nc.sync.dma_start(out=SQ[:, :, :, KK], in_=msk)
        nc.sync.dma_start(out=ID, in_=idn)

        # A_frozen[i, n] = sum_j (-4C)[j, i] * Vfz[j, n], done once.
        for ib in range(NB):
            GF = psum.tile([P, NBK], f32, tag="G")
            for jc in range(NB):
                nc.tensor.matmul(
                    GF,
                    CN[:, jc, bass.ts(ib, P)],
                    VFZ[:, jc, :],
                    start=(jc == 0),
                    stop=(jc == NB - 1),
                )
            nc.scalar.copy(out=AF[:, ib, :], in_=GF)

        for s in range(sweeps):
            last_sweep = s == sweeps - 1
            for ib in range(NB):
                G = psum.tile([P, NBK], f32, tag="G")
                G3 = G.rearrange("p (b k) -> p b k", b=BL)
                # G = A_frozen + (-4C)[rows, :] @ Vhat, chunk ib-1 (the one
                # written by the previous block) last.
                nc.tensor.matmul(G, ID, AF[:, ib, :], start=True, stop=False)
                for u in range(NB):
                    jc = (ib + u) % NB
                    nc.tensor.matmul(
                        G,
                        CN[:, jc, bass.ts(ib, P)],
                        VH[:, jc, :],
                        start=False,
                        stop=(u == NB - 1),
                    )
                xs = [VH[:, ib, :]]
                for m in range(1, M + 1):
                    if m >= 2:
                        # G += (-4L) @ x^{m-1} - (-4L) @ x^{m-2}; the group was
                        # closed before the m-1 read, so skip the group check.
                        nc.tensor.matmul(
                            G,
                            U4N[:, ib, :],
                            xs[m - 2],
                            start=False,
                            stop=False,
                            skip_group_check=True,
                        )
                        nc.tensor.matmul(
                            G,
                            U4[:, ib, :],
                            xs[m - 1],
                            start=False,
                            stop=True,
                            skip_group_check=True,
                        )
                    nc.scalar.square(out=SQ[:, ib, :, 0:KK], in_=G3)
                    ss = stat.tile([P, BL], f32, tag="ss")
                    nc.vector.reduce_sum(
                        out=ss, in_=SQ[:, ib, :, :], axis=mybir.AxisListType.X
                    )
                    rs = stat.tile([P, BL], f32, tag="rs")
                    if RSQRT_ONE_OP:
                        # rs = 1/sqrt(|ss|)
                        nc.scalar.activation(
                            out=rs,
                            in_=ss,
                            func=mybir.ActivationFunctionType.Abs_reciprocal_sqrt,
                        )
                    else:
                        sn = stat.tile([P, BL], f32, tag="sn")
                        nc.scalar.sqrt(out=sn, in_=ss)
                        nc.vector.reciprocal(out=rs, in_=sn)
                    if m == M:
                        tgt = VH[:, ib, :]
                    else:
                        tgt = xbp.tile([P, NBK], f32, tag="xb")
                    nc.vector.tensor_mul(
                        out=tgt.rearrange("p (b k) -> p b k", b=BL),
                        in0=G3,
                        in1=rs[:, :, None].broadcast_to([P, BL, KK]),
                    )
                    xs.append(tgt)
                if last_sweep:
                    nc.sync.dma_start(out=vout_t[:, ib, :], in_=VH[:, ib, :])
    nc.finalize()
    return nc


_INIT_SRC = r"""
import sys
import numpy as np
import jax
import jax.numpy as jnp

AUX, NV, KK = 255, 1024, 32
inp = np.load(sys.argv[1])
z = jnp.asarray(inp["z"])
ii = jnp.asarray(inp["ii"])
b = z.shape[0]
z_full = jnp.concatenate(
    [jnp.ones((b, 1), z.dtype), z, jnp.zeros((b, AUX), z.dtype)], axis=1
)
iif = jnp.concatenate(
    [jnp.ones((b, 1), ii.dtype), ii, jnp.zeros((b, AUX), ii.dtype)], axis=1
)
k1, k2 = jax.random.split(jax.random.key(42))
v0 = jax.random.normal(k1, (b, KK), jnp.float32)
v0 = v0 / jnp.linalg.norm(v0, axis=-1, keepdims=True)
R = jax.random.normal(k2, (b, NV, KK), jnp.float32)
R = R - jnp.einsum("bnk,bk->bn", R, v0)[..., None] * v0[:, None, :]
R = R / jnp.linalg.norm(R, axis=-1, keepdims=True)
c = jnp.cos(jnp.pi * z_full)[..., None]
s = jnp.sin(jnp.pi * z_full)[..., None]
V = jnp.where(iif[..., None] > 0, -c * v0[:, None, :] + s * R, R)
V = V.at[:, 0].set(v0)
np.savez(
    sys.argv[2],
    V0=np.asarray(V),
    v0=np.asarray(v0),
    z_full=np.asarray(z_full),
    free=np.asarray(iif == 0),
)
"""


def _host_init(z, is_input):
    """Replicates reference._build_full + _init_V with CPU jax.

    The reference oracle runs on single-device (CPU) jax; jax.random on the
    neuron backend yields different draws, so the init is computed in a
    subprocess pinned to JAX_PLATFORMS=cpu (this process needs the
    accelerator backend for the bass kernel and can't switch platforms).
    """
    import os
    import subprocess
    import sys
    import tempfile

    with tempfile.TemporaryDirectory() as td:
        inp_path = os.path.join(td, "inp.npz")
        out_path = os.path.join(td, "out.npz")
        np.savez(inp_path, z=z, ii=is_input)
        env = dict(os.environ)
        env["JAX_PLATFORMS"] = "cpu"
        # Blanking the axon boot gate skips accelerator registration in the
        # child; hand it this process's live sys.path so jax/numpy resolve
        # without the sitecustomize chain.
        env.pop("TRN_TERMINAL_POOL_IPS", None)
        env["PYTHONPATH"] = os.pathsep.join(p for p in sys.path if p)
        r = subprocess.run(
            [sys.executable, "-c", _INIT_SRC, inp_path, out_path],
            env=env,
            capture_output=True,
            text=True,
        )
        if r.returncode == 0 and os.path.exists(out_path):
            d = np.load(out_path)
            return d["V0"], d["v0"], d["z_full"], d["free"]
        sys.stderr.write(
            "kernel.py: CPU-jax init subprocess failed, falling back to "
            f"in-process jax\n{r.stderr[-2000:]}\n"
        )
    return _host_init_inproc(z, is_input)


def _host_init_inproc(z, is_input):
    """In-process fallback (whatever jax backend is active)."""
    import jax
    import jax.numpy as jnp

    b = z.shape[0]
    z_ = jnp.asarray(z)
    ii_ = jnp.asarray(is_input)
    z_full = jnp.concatenate(
        [jnp.ones((b, 1), z_.dtype), z_, jnp.zeros((b, AUX), z_.dtype)], axis=1
    )
    iif = jnp.concatenate(
        [jnp.ones((b, 1), ii_.dtype), ii_, jnp.zeros((b, AUX), ii_.dtype)], axis=1
    )
    k1, k2 = jax.random.split(jax.random.key(42))
    v0 = jax.random.normal(k1, (b, KK), jnp.float32)
    v0 = v0 / jnp.linalg.norm(v0, axis=-1, keepdims=True)
    R = jax.random.normal(k2, (b, NV, KK), jnp.float32)
    R = R - jnp.einsum("bnk,bk->bn", R, v0)[..., None] * v0[:, None, :]
    R = R / jnp.linalg.norm(R, axis=-1, keepdims=True)
    c = jnp.cos(jnp.pi * z_full)[..., None]
    s = jnp.sin(jnp.pi * z_full)[..., None]
    V = jnp.where(iif[..., None] > 0, -c * v0[:, None, :] + s * R, R)
    V = V.at[:, 0].set(v0)
    return (
        np.asarray(V),
        np.asarray(v0),
        np.asarray(z_full),
        np.asarray(iif == 0),
    )


TRACE = False
LAST_RESULTS = None


def kernel(C, z, is_input):
    global LAST_RESULTS
    from concourse.bass_utils import run_bass_kernel_spmd

    C = np.ascontiguousarray(np.asarray(C, np.float32))
    z = np.asarray(z, np.float32)
    is_input = np.asarray(is_input)

    V0, v0, z_full, free = _host_init(z, is_input)

    CN4 = np.ascontiguousarray(-4.0 * C)
    U4 = np.stack(
        [np.triu(CN4[b * P : (b + 1) * P, b * P : (b + 1) * P], 1) for b in range(NB)]
    ).astype(np.float32)
    U4N = np.ascontiguousarray(-U4)
    IDN = np.eye(P, dtype=np.float32)

    in_maps = []
    pre = []
    for c in range(NCORES):
        bc = slice(c * BL, (c + 1) * BL)
        V0c = V0[bc]            # (BL, NV, KK)
        freec = free[bc]        # (BL, NV)
        freeT = freec.T         # (NV, BL)
        Vt = np.ascontiguousarray(V0c.transpose(1, 0, 2))  # (NV, BL, KK)
        vh0 = np.where(freeT[:, :, None], Vt, 0.0).reshape(NV, NBK)
        vfz = np.where(~freeT[:, :, None], Vt, 0.0).reshape(NV, NBK)
        mskc = (
            (BIGMASK * (1.0 - freeT.astype(np.float32)))
            .reshape(NB, P, BL)
            .transpose(1, 0, 2)
        )
        in_maps.append(
            dict(
                cn4=CN4,
                u4=U4,
                u4n=U4N,
                vh0=np.ascontiguousarray(vh0.astype(np.float32)),
                vfz=np.ascontiguousarray(vfz.astype(np.float32)),
                msk=np.ascontiguousarray(mskc.astype(np.float32)),
                idn=IDN,
            )
        )
        pre.append((V0c, freec))

    if "nc" not in _CACHE:
        _CACHE["nc"] = _build(M_INNER, SWEEPS)
    nc = _CACHE["nc"]

    res = run_bass_kernel_spmd(
        nc, in_maps, core_ids=list(range(NCORES)), trace=TRACE
    )
    LAST_RESULTS = res

    Vfull = np.zeros((B, NV, KK), np.float32)
    for c in range(NCORES):
        V0c, freec = pre[c]
        vo = res.results[c]["vout"]  # (NV, NBK)
        Vdev = vo.reshape(NV, BL, KK).transpose(1, 0, 2)
        Vfull[c * BL : (c + 1) * BL] = np.where(freec[:, :, None], Vdev, V0c)

    cosang = np.clip(
        -np.einsum("bnk,bk->bn", Vfull, v0), -1.0 + 1e-7, 1.0 - 1e-7
    )
    zo = np.where(free, np.arccos(cosang) / np.pi, z_full)
    return np.ascontiguousarray(zo[:, 1 : NV - AUX].astype(np.float32))


# revision 12
# speedup vs baseline: 275.6224x; 275.6224x over previous
"""Trainium2 Bass kernel for nn_MixNet (Mixing-method coordinate descent).

Sharding: data-parallel over batch B=64 across 8 NeuronCores (8 batch
elements per core); C is replicated. Each core runs 10 sweeps of block
coordinate descent over the 1024 coordinates in reference order
(8 blocks of 128), where the sequential within-block Gauss-Seidel update
is reproduced by M_INNER fixed-point iterations of

    x^m = normalize(A + L_blk @ (x^{m-1} - x^0))

which converges geometrically (~0.3x error per iteration) to the exact
sequential result. All heavy work is tensor-engine matmuls plus a
[128 x 256]-wide vectorized normalize per iteration.

Device layout (per core):
  V-hat [1024, 256]: free entries of V (coordinate-major; 256 = 8 batch x 32 K),
      frozen (is_input) entries are zero; their contribution to the matvec is a
      constant A_frozen = (-4C) @ V_frozen, computed once and re-seeded into
      PSUM each block via an identity matmul.
  All matmuls carry a factor of -4 folded into C so that the per-row scale is
  exactly rs = Dsqrt(0.25 * sum_k G^2) = 1/||G|| with the normalize sign
  (v = -g/||g||) absorbed, and a +1e30 mask column appended to the squared
  tile makes frozen rows come out ~1e-15 (i.e. zero) without a select op.
"""

import numpy as np

N_IN, AUX = 768, 255
NV = N_IN + 1 + AUX          # 1024 variables
KK = 32                      # embedding dim K
B = 64                       # full batch
NCORES = 8
BL = B // NCORES             # batch per core = 8
NBK = BL * KK                # 256 free width per core
P = 128                      # partitions / block size
NB = NV // P                 # 8 coordinate blocks
SWEEPS = 10
M_INNER = 3                  # fixed-point iterations per block
BIGMASK = 1e30
RSQRT_ONE_OP = False         # Abs_reciprocal_sqrt is inaccurate on HW; use Sqrt+reciprocal
# float32r matmuls (1 cyc/row vs 4 for fp32) are rejected by birverifier here:
# every producer of a PE input (DMAs, DVE scale writes) would have to declare
# fp32r-rounded output. Left off; fp32 numerics validated end-to-end.
MM_F32R = False

_CACHE = {}


def _build(M=M_INNER, sweeps=SWEEPS):
    from contextlib import ExitStack

    import concourse.bass as bass
    import concourse.mybir as mybir
    import concourse.tile as tile
    from concourse import bacc

    f32 = mybir.dt.float32

    def mmt(ap):
        # PE input dtype for matmuls: float32r streams 1 row/cycle at N>=256
        return ap.bitcast(mybir.dt.float32r) if MM_F32R else ap

    nc = bacc.Bacc(
        "TRN2", target_bir_lowering=False, debug=False, enable_asserts=False
    )
    cn4 = nc.dram_tensor("cn4", [NV, NV], f32, kind="ExternalInput").ap()
    u4 = nc.dram_tensor("u4", [NB, P, P], f32, kind="ExternalInput").ap()
    u4n = nc.dram_tensor("u4n", [NB, P, P], f32, kind="ExternalInput").ap()
    vh0 = nc.dram_tensor("vh0", [NV, NBK], f32, kind="ExternalInput").ap()
    vfz = nc.dram_tensor("vfz", [NV, NBK], f32, kind="ExternalInput").ap()
    msk = nc.dram_tensor("msk", [P, NB, BL], f32, kind="ExternalInput").ap()
    idn = nc.dram_tensor("idn", [P, P], f32, kind="ExternalInput").ap()
    vout = nc.dram_tensor("vout", [NV, NBK], f32, kind="ExternalOutput").ap()
    vout_t = vout.rearrange("(jc p) n -> p jc n", p=P)

    with tile.TileContext(nc) as tc, ExitStack() as ctx:
        const = ctx.enter_context(tc.tile_pool(name="const", bufs=1))
        stat = ctx.enter_context(tc.tile_pool(name="stat", bufs=4))
        xbp = ctx.enter_context(tc.tile_pool(name="xb", bufs=3))
        psum = ctx.enter_context(tc.tile_pool(name="psum", bufs=2, space="PSUM"))

        CN = const.tile([P, NB, NV], f32)
        U4 = const.tile([P, NB, P], f32)
        U4N = const.tile([P, NB, P], f32)
        VH = const.tile([P, NB, NBK], f32)
        VFZ = const.tile([P, NB, NBK], f32)
        AF = const.tile([P, NB, NBK], f32)
        SQ = const.tile([P, NB, BL, KK + 1], f32)
        ID = const.tile([P, P], f32)

        nc.sync.dma_start(out=CN, in_=cn4.rearrange("(jc p) i -> p jc i", p=P))
        nc.sync.dma_start(out=U4, in_=u4.rearrange("b j i -> j b i"))
        nc.sync.dma_start(out=U4N, in_=u4n.rearrange("b j i -> j b i"))
        nc.sync.dma_start(out=VH, in_=vh0.rearrange("(jc p) n -> p jc n", p=P))
        nc.sync.dma_start(out=VFZ, in_=vfz.rearrange("(jc p) n -> p jc n", p=P))
        nc.sync.dma_start(out=SQ[:, :, :, KK], in_=msk)
        nc.sync.dma_start(out=ID, in_=idn)

        # A_frozen[i, n] = sum_j (-4C)[j, i] * Vfz[j, n], done once.
        for ib in range(NB):
            GF = psum.tile([P, NBK], f32, tag="G")
            for jc in range(NB):
                nc.tensor.matmul(
                    GF,
                    CN[:, jc, bass.ts(ib, P)],
                    VFZ[:, jc, :],
                    start=(jc == 0),
                    stop=(jc == NB - 1),
                )
            nc.scalar.copy(out=AF[:, ib, :], in_=GF)

        for s in range(sweeps):
            last_sweep = s == sweeps - 1
            for ib in range(NB):
                G = psum.tile([P, NBK], f32, tag="G")
                G3 = G.rearrange("p (b k) -> p b k", b=BL)
                # G = A_frozen + (-4C)[rows, :] @ Vhat, chunk ib-1 (the one
                # written by the previous block) last.
                nc.tensor.matmul(G, ID, AF[:, ib, :], start=True, stop=False)
                for u in range(NB):
                    jc = (ib + u) % NB
                    nc.tensor.matmul(
                        G,
                        mmt(CN[:, jc, bass.ts(ib, P)]),
                        mmt(VH[:, jc, :]),
                        start=False,
                        stop=(u == NB - 1),
                    )
                xs = [VH[:, ib, :]]
                for m in range(1, M + 1):
                    if m >= 2:
                        # G += (-4L) @ x^{m-1} - (-4L) @ x^{m-2}; the group was
                        # closed before the m-1 read, so skip the group check.
                        nc.tensor.matmul(
                            G,
                            mmt(U4N[:, ib, :]),
                            mmt(xs[m - 2]),
                            start=False,
                            stop=False,
                            skip_group_check=True,
                        )
                        nc.tensor.matmul(
                            G,
                            mmt(U4[:, ib, :]),
                            mmt(xs[m - 1]),
                            start=False,
                            stop=True,
                            skip_group_check=True,
                        )
                    nc.scalar.square(out=SQ[:, ib, :, 0:KK], in_=G3)
                    ss = stat.tile([P, BL], f32, tag="ss")
                    nc.vector.reduce_sum(
                        out=ss, in_=SQ[:, ib, :, :], axis=mybir.AxisListType.X
                    )
                    rs = stat.tile([P, BL], f32, tag="rs")
                    if RSQRT_ONE_OP:
                        # rs = 1/sqrt(|ss|)
                        nc.scalar.activation(
                            out=rs,
                            in_=ss,
                            func=mybir.ActivationFunctionType.Abs_reciprocal_sqrt,
                        )
                    else:
                        sn = stat.tile([P, BL], f32, tag="sn")
                        nc.scalar.sqrt(out=sn, in_=ss)
                        nc.vector.reciprocal(out=rs, in_=sn)
                    if m == M:
                        tgt = VH[:, ib, :]
                    else:
                        tgt = xbp.tile([P, NBK], f32, tag="xb")
                    nc.vector.tensor_mul(
                        out=tgt.rearrange("p (b k) -> p b k", b=BL),
                        in0=G3,
                        in1=rs[:, :, None].broadcast_to([P, BL, KK]),
                    )
                    xs.append(tgt)
                if last_sweep:
                    nc.sync.dma_start(out=vout_t[:, ib, :], in_=VH[:, ib, :])
    nc.finalize()
    return nc


_INIT_SRC = r"""
import sys
import numpy as np
import jax
import jax.numpy as jnp

AUX, NV, KK = 255, 1024, 32
inp = np.load(sys.argv[1])
z = jnp.asarray(inp["z"])
ii = jnp.asarray(inp["ii"])
b = z.shape[0]
z_full = jnp.concatenate(
    [jnp.ones((b, 1), z.dtype), z, jnp.zeros((b, AUX), z.dtype)], axis=1
)
iif = jnp.concatenate(
    [jnp.ones((b, 1), ii.dtype), ii, jnp.zeros((b, AUX), ii.dtype)], axis=1
)
k1, k2 = jax.random.split(jax.random.key(42))
v0 = jax.random.normal(k1, (b, KK), jnp.float32)
v0 = v0 / jnp.linalg.norm(v0, axis=-1, keepdims=True)
R = jax.random.normal(k2, (b, NV, KK), jnp.float32)
R = R - jnp.einsum("bnk,bk->bn", R, v0)[..., None] * v0[:, None, :]
R = R / jnp.linalg.norm(R, axis=-1, keepdims=True)
c = jnp.cos(jnp.pi * z_full)[..., None]
s = jnp.sin(jnp.pi * z_full)[..., None]
V = jnp.where(iif[..., None] > 0, -c * v0[:, None, :] + s * R, R)
V = V.at[:, 0].set(v0)
np.savez(
    sys.argv[2],
    V0=np.asarray(V),
    v0=np.asarray(v0),
    z_full=np.asarray(z_full),
    free=np.asarray(iif == 0),
)
"""


def _host_init(z, is_input):
    """Replicates reference._build_full + _init_V with CPU jax.

    The reference oracle runs on single-device (CPU) jax; jax.random on the
    neuron backend yields different draws, so the init is computed in a
    subprocess pinned to JAX_PLATFORMS=cpu (this process needs the
    accelerator backend for the bass kernel and can't switch platforms).
    """
    import os
    import subprocess
    import sys
    import tempfile

    with tempfile.TemporaryDirectory() as td:
        inp_path = os.path.join(td, "inp.npz")
        out_path = os.path.join(td, "out.npz")
        np.savez(inp_path, z=z, ii=is_input)
        env = dict(os.environ)
        env["JAX_PLATFORMS"] = "cpu"
        # Blanking the axon boot gate skips accelerator registration in the
        # child; hand it this process's live sys.path so jax/numpy resolve
        # without the sitecustomize chain.
        env.pop("TRN_TERMINAL_POOL_IPS", None)
        env["PYTHONPATH"] = os.pathsep.join(p for p in sys.path if p)
        r = subprocess.run(
            [sys.executable, "-c", _INIT_SRC, inp_path, out_path],
            env=env,
            capture_output=True,
            text=True,
        )
        if r.returncode == 0 and os.path.exists(out_path):
            d = np.load(out_path)
            return d["V0"], d["v0"], d["z_full"], d["free"]
        sys.stderr.write(
            "kernel.py: CPU-jax init subprocess failed, falling back to "
            f"in-process jax\n{r.stderr[-2000:]}\n"
        )
    return _host_init_inproc(z, is_input)


def _host_init_inproc(z, is_input):
    """In-process fallback (whatever jax backend is active)."""
    import jax
    import jax.numpy as jnp

    b = z.shape[0]
    z_ = jnp.asarray(z)
    ii_ = jnp.asarray(is_input)
    z_full = jnp.concatenate(
        [jnp.ones((b, 1), z_.dtype), z_, jnp.zeros((b, AUX), z_.dtype)], axis=1
    )
    iif = jnp.concatenate(
        [jnp.ones((b, 1), ii_.dtype), ii_, jnp.zeros((b, AUX), ii_.dtype)], axis=1
    )
    k1, k2 = jax.random.split(jax.random.key(42))
    v0 = jax.random.normal(k1, (b, KK), jnp.float32)
    v0 = v0 / jnp.linalg.norm(v0, axis=-1, keepdims=True)
    R = jax.random.normal(k2, (b, NV, KK), jnp.float32)
    R = R - jnp.einsum("bnk,bk->bn", R, v0)[..., None] * v0[:, None, :]
    R = R / jnp.linalg.norm(R, axis=-1, keepdims=True)
    c = jnp.cos(jnp.pi * z_full)[..., None]
    s = jnp.sin(jnp.pi * z_full)[..., None]
    V = jnp.where(iif[..., None] > 0, -c * v0[:, None, :] + s * R, R)
    V = V.at[:, 0].set(v0)
    return (
        np.asarray(V),
        np.asarray(v0),
        np.asarray(z_full),
        np.asarray(iif == 0),
    )


TRACE = False
LAST_RESULTS = None


def kernel(C, z, is_input):
    global LAST_RESULTS
    from concourse.bass_utils import run_bass_kernel_spmd

    C = np.ascontiguousarray(np.asarray(C, np.float32))
    z = np.asarray(z, np.float32)
    is_input = np.asarray(is_input)

    V0, v0, z_full, free = _host_init(z, is_input)

    CN4 = np.ascontiguousarray(-4.0 * C)
    U4 = np.stack(
        [np.triu(CN4[b * P : (b + 1) * P, b * P : (b + 1) * P], 1) for b in range(NB)]
    ).astype(np.float32)
    U4N = np.ascontiguousarray(-U4)
    IDN = np.eye(P, dtype=np.float32)

    in_maps = []
    pre = []
    for c in range(NCORES):
        bc = slice(c * BL, (c + 1) * BL)
        V0c = V0[bc]            # (BL, NV, KK)
        freec = free[bc]        # (BL, NV)
        freeT = freec.T         # (NV, BL)
        Vt = np.ascontiguousarray(V0c.transpose(1, 0, 2))  # (NV, BL, KK)
        vh0 = np.where(freeT[:, :, None], Vt, 0.0).reshape(NV, NBK)
        vfz = np.where(~freeT[:, :, None], Vt, 0.0).reshape(NV, NBK)
        mskc = (
            (BIGMASK * (1.0 - freeT.astype(np.float32)))
            .reshape(NB, P, BL)
            .transpose(1, 0, 2)
        )
        in_maps.append(
            dict(
                cn4=CN4,
                u4=U4,
                u4n=U4N,
                vh0=np.ascontiguousarray(vh0.astype(np.float32)),
                vfz=np.ascontiguousarray(vfz.astype(np.float32)),
                msk=np.ascontiguousarray(mskc.astype(np.float32)),
                idn=IDN,
            )
        )
        pre.append((V0c, freec))

    if "nc" not in _CACHE:
        _CACHE["nc"] = _build(M_INNER, SWEEPS)
    nc = _CACHE["nc"]

    res = run_bass_kernel_spmd(
        nc, in_maps, core_ids=list(range(NCORES)), trace=TRACE
    )
    LAST_RESULTS = res

    Vfull = np.zeros((B, NV, KK), np.float32)
    for c in range(NCORES):
        V0c, freec = pre[c]
        vo = res.results[c]["vout"]  # (NV, NBK)
        Vdev = vo.reshape(NV, BL, KK).transpose(1, 0, 2)
        Vfull[c * BL : (c + 1) * BL] = np.where(freec[:, :, None], Vdev, V0c)

    cosang = np.clip(
        -np.einsum("bnk,bk->bn", Vfull, v0), -1.0 + 1e-7, 1.0 - 1e-7
    )
    zo = np.where(free, np.arccos(cosang) / np.pi, z_full)
    return np.ascontiguousarray(zo[:, 1 : NV - AUX].astype(np.float32))


# revision 15
# speedup vs baseline: 337.5471x; 1.2247x over previous
"""Trainium2 Bass kernel for nn_MixNet (Mixing-method coordinate descent).

Sharding: data-parallel over batch B=64 across 8 NeuronCores (8 batch
elements per core); C is replicated. Each core runs 10 sweeps of block
coordinate descent over the 1024 coordinates in reference order
(8 blocks of 128), where the sequential within-block Gauss-Seidel update
is reproduced by M_INNER fixed-point iterations of

    x^m = normalize(A + L_blk @ (x^{m-1} - x^0))

which converges geometrically (~0.3x error per iteration) to the exact
sequential result. All heavy work is tensor-engine matmuls plus a
[128 x 256]-wide vectorized normalize per iteration.

Device layout (per core):
  V-hat [1024, 256]: free entries of V (coordinate-major; 256 = 8 batch x 32 K),
      frozen (is_input) entries are zero; their contribution to the matvec is a
      constant A_frozen = (-4C) @ V_frozen, computed once and re-seeded into
      PSUM each block via an identity matmul.
  All matmuls carry a factor of -4 folded into C so that the per-row scale is
  exactly rs = Dsqrt(0.25 * sum_k G^2) = 1/||G|| with the normalize sign
  (v = -g/||g||) absorbed, and a +1e30 mask column appended to the squared
  tile makes frozen rows come out ~1e-15 (i.e. zero) without a select op.
"""

import numpy as np

N_IN, AUX = 768, 255
NV = N_IN + 1 + AUX          # 1024 variables
KK = 32                      # embedding dim K
B = 64                       # full batch
NCORES = 8
BL = B // NCORES             # batch per core = 8
NBK = BL * KK                # 256 free width per core
P = 128                      # partitions / block size
NB = NV // P                 # 8 coordinate blocks
SWEEPS = 10
M_INNER = 3                  # fixed-point iterations per block (when no schedule)
# Per-sweep inner-iteration counts: error injected in the FIRST sweep
# dominates the final deviation, so spend iterations there. Measured vs the
# reference: maxabs 6.6e-4 / rel-l2 2.4e-4 at 192 block-steps (uniform M=3:
# 1.45e-3 / 4.75e-4 at 240 steps).
M_SCHEDULE = [4, 3, 2, 2, 2, 2, 2, 2, 2, 3]
BIGMASK = 1e30
RSQRT_ONE_OP = False         # Abs_reciprocal_sqrt is inaccurate on HW; use Sqrt+reciprocal
# float32r matmuls (1 cyc/row vs 4 for fp32) are rejected by birverifier here:
# every producer of a PE input (DMAs, DVE scale writes) would have to declare
# fp32r-rounded output. Left off; fp32 numerics validated end-to-end.
MM_F32R = False

_CACHE = {}


def _build(M=None, sweeps=SWEEPS):
    Ms = ([M] * sweeps if M is not None else (M_SCHEDULE + [M_INNER] * sweeps)[:sweeps])
    from contextlib import ExitStack

    import concourse.bass as bass
    import concourse.mybir as mybir
    import concourse.tile as tile
    from concourse import bacc

    f32 = mybir.dt.float32

    def mmt(ap):
        # PE input dtype for matmuls: float32r streams 1 row/cycle at N>=256
        return ap.bitcast(mybir.dt.float32r) if MM_F32R else ap

    nc = bacc.Bacc(
        "TRN2", target_bir_lowering=False, debug=False, enable_asserts=False
    )
    cn4 = nc.dram_tensor("cn4", [NV, NV], f32, kind="ExternalInput").ap()
    u4 = nc.dram_tensor("u4", [NB, P, P], f32, kind="ExternalInput").ap()
    u4n = nc.dram_tensor("u4n", [NB, P, P], f32, kind="ExternalInput").ap()
    vh0 = nc.dram_tensor("vh0", [NV, NBK], f32, kind="ExternalInput").ap()
    vfz = nc.dram_tensor("vfz", [NV, NBK], f32, kind="ExternalInput").ap()
    msk = nc.dram_tensor("msk", [P, NB, BL], f32, kind="ExternalInput").ap()
    idn = nc.dram_tensor("idn", [P, P], f32, kind="ExternalInput").ap()
    vout = nc.dram_tensor("vout", [NV, NBK], f32, kind="ExternalOutput").ap()
    vout_t = vout.rearrange("(jc p) n -> p jc n", p=P)

    with tile.TileContext(nc) as tc, ExitStack() as ctx:
        const = ctx.enter_context(tc.tile_pool(name="const", bufs=1))
        stat = ctx.enter_context(tc.tile_pool(name="stat", bufs=4))
        xbp = ctx.enter_context(tc.tile_pool(name="xb", bufs=3))
        psum = ctx.enter_context(tc.tile_pool(name="psum", bufs=3, space="PSUM"))

        CN = const.tile([P, NB, NV], f32)
        U4 = const.tile([P, NB, P], f32)
        U4N = const.tile([P, NB, P], f32)
        VH = const.tile([P, NB, NBK], f32)
        VFZ = const.tile([P, NB, NBK], f32)
        AF = const.tile([P, NB, NBK], f32)
        SQ = const.tile([P, NB, BL, KK + 1], f32)
        ID = const.tile([P, P], f32)

        nc.sync.dma_start(out=CN, in_=cn4.rearrange("(jc p) i -> p jc i", p=P))
        nc.sync.dma_start(out=U4, in_=u4.rearrange("b j i -> j b i"))
        nc.sync.dma_start(out=U4N, in_=u4n.rearrange("b j i -> j b i"))
        nc.sync.dma_start(out=VH, in_=vh0.rearrange("(jc p) n -> p jc n", p=P))
        nc.sync.dma_start(out=VFZ, in_=vfz.rearrange("(jc p) n -> p jc n", p=P))
        nc.sync.dma_start(out=SQ[:, :, :, KK], in_=msk)
        nc.sync.dma_start(out=ID, in_=idn)

        # A_frozen[i, n] = sum_j (-4C)[j, i] * Vfz[j, n], done once.
        for ib in range(NB):
            GF = psum.tile([P, NBK], f32, tag="G")
            for jc in range(NB):
                nc.tensor.matmul(
                    GF,
                    CN[:, jc, bass.ts(ib, P)],
                    VFZ[:, jc, :],
                    start=(jc == 0),
                    stop=(jc == NB - 1),
                )
            nc.scalar.copy(out=AF[:, ib, :], in_=GF)

        # Software-pipelined emission: the PE is in-order, so each block's
        # seed + 7 independent chunk matmuls are emitted during the PREVIOUS
        # block (they only read Vhat tiles finalized at least one block ago),
        # keeping the PE busy under the previous block's normalize chain.
        # Only the chunk jc = ib-1 (written by the immediately preceding
        # block's final scale) is emitted at block start.
        def emit_early(ib):
            """Seed + the 7 chunk matmuls that don't read Vhat[ib-1]."""
            G = psum.tile([P, NBK], f32, tag="G")
            nc.tensor.matmul(G, ID, AF[:, ib, :], start=True, stop=False)
            for u in range(NB - 1):
                jc = (ib + u) % NB
                nc.tensor.matmul(
                    G,
                    mmt(CN[:, jc, bass.ts(ib, P)]),
                    mmt(VH[:, jc, :]),
                    start=False,
                    stop=False,
                )
            return G

        G_next = emit_early(0)
        for s in range(sweeps):
            last_sweep = s == sweeps - 1
            M = Ms[s]
            for ib in range(NB):
                G = G_next
                G3 = G.rearrange("p (b k) -> p b k", b=BL)
                jc = (ib + NB - 1) % NB
                nc.tensor.matmul(
                    G,
                    mmt(CN[:, jc, bass.ts(ib, P)]),
                    mmt(VH[:, jc, :]),
                    start=False,
                    stop=True,
                )
                if not (last_sweep and ib == NB - 1):
                    G_next = emit_early((ib + 1) % NB)
                xs = [VH[:, ib, :]]
                for m in range(1, M + 1):
                    if m >= 2:
                        # G += (-4L) @ x^{m-1} - (-4L) @ x^{m-2}; the group was
                        # closed before the m-1 read, so skip the group check.
                        nc.tensor.matmul(
                            G,
                            mmt(U4N[:, ib, :]),
                            mmt(xs[m - 2]),
                            start=False,
                            stop=False,
                            skip_group_check=True,
                        )
                        nc.tensor.matmul(
                            G,
                            mmt(U4[:, ib, :]),
                            mmt(xs[m - 1]),
                            start=False,
                            stop=True,
                            skip_group_check=True,
                        )
                    nc.scalar.square(out=SQ[:, ib, :, 0:KK], in_=G3)
                    ss = stat.tile([P, BL], f32, tag="ss")
                    nc.vector.reduce_sum(
                        out=ss, in_=SQ[:, ib, :, :], axis=mybir.AxisListType.X
                    )
                    rs = stat.tile([P, BL], f32, tag="rs")
                    if RSQRT_ONE_OP:
                        # rs = 1/sqrt(|ss|)
                        nc.scalar.activation(
                            out=rs,
                            in_=ss,
                            func=mybir.ActivationFunctionType.Abs_reciprocal_sqrt,
                        )
                    else:
                        sn = stat.tile([P, BL], f32, tag="sn")
                        nc.scalar.sqrt(out=sn, in_=ss)
                        nc.vector.reciprocal(out=rs, in_=sn)
                    if m == M:
                        tgt = VH[:, ib, :]
                    else:
                        tgt = xbp.tile([P, NBK], f32, tag="xb")
                    nc.vector.tensor_mul(
                        out=tgt.rearrange("p (b k) -> p b k", b=BL),
                        in0=G3,
                        in1=rs[:, :, None].broadcast_to([P, BL, KK]),
                    )
                    xs.append(tgt)
                if last_sweep:
                    nc.sync.dma_start(out=vout_t[:, ib, :], in_=VH[:, ib, :])
    nc.finalize()
    return nc


_INIT_SRC = r"""
import sys
import numpy as np
import jax
import jax.numpy as jnp

AUX, NV, KK = 255, 1024, 32
inp = np.load(sys.argv[1])
z = jnp.asarray(inp["z"])
ii = jnp.asarray(inp["ii"])
b = z.shape[0]
z_full = jnp.concatenate(
    [jnp.ones((b, 1), z.dtype), z, jnp.zeros((b, AUX), z.dtype)], axis=1
)
iif = jnp.concatenate(
    [jnp.ones((b, 1), ii.dtype), ii, jnp.zeros((b, AUX), ii.dtype)], axis=1
)
k1, k2 = jax.random.split(jax.random.key(42))
v0 = jax.random.normal(k1, (b, KK), jnp.float32)
v0 = v0 / jnp.linalg.norm(v0, axis=-1, keepdims=True)
R = jax.random.normal(k2, (b, NV, KK), jnp.float32)
R = R - jnp.einsum("bnk,bk->bn", R, v0)[..., None] * v0[:, None, :]
R = R / jnp.linalg.norm(R, axis=-1, keepdims=True)
c = jnp.cos(jnp.pi * z_full)[..., None]
s = jnp.sin(jnp.pi * z_full)[..., None]
V = jnp.where(iif[..., None] > 0, -c * v0[:, None, :] + s * R, R)
V = V.at[:, 0].set(v0)
np.savez(
    sys.argv[2],
    V0=np.asarray(V),
    v0=np.asarray(v0),
    z_full=np.asarray(z_full),
    free=np.asarray(iif == 0),
)
"""


def _host_init(z, is_input):
    """Replicates reference._build_full + _init_V with CPU jax.

    The reference oracle runs on single-device (CPU) jax; jax.random on the
    neuron backend yields different draws, so the init is computed in a
    subprocess pinned to JAX_PLATFORMS=cpu (this process needs the
    accelerator backend for the bass kernel and can't switch platforms).
    """
    import os
    import subprocess
    import sys
    import tempfile

    with tempfile.TemporaryDirectory() as td:
        inp_path = os.path.join(td, "inp.npz")
        out_path = os.path.join(td, "out.npz")
        np.savez(inp_path, z=z, ii=is_input)
        env = dict(os.environ)
        env["JAX_PLATFORMS"] = "cpu"
        # Blanking the axon boot gate skips accelerator registration in the
        # child; hand it this process's live sys.path so jax/numpy resolve
        # without the sitecustomize chain.
        env.pop("TRN_TERMINAL_POOL_IPS", None)
        env["PYTHONPATH"] = os.pathsep.join(p for p in sys.path if p)
        r = subprocess.run(
            [sys.executable, "-c", _INIT_SRC, inp_path, out_path],
            env=env,
            capture_output=True,
            text=True,
        )
        if r.returncode == 0 and os.path.exists(out_path):
            d = np.load(out_path)
            return d["V0"], d["v0"], d["z_full"], d["free"]
        sys.stderr.write(
            "kernel.py: CPU-jax init subprocess failed, falling back to "
            f"in-process jax\n{r.stderr[-2000:]}\n"
        )
    return _host_init_inproc(z, is_input)


def _host_init_inproc(z, is_input):
    """In-process fallback (whatever jax backend is active)."""
    import jax
    import jax.numpy as jnp

    b = z.shape[0]
    z_ = jnp.asarray(z)
    ii_ = jnp.asarray(is_input)
    z_full = jnp.concatenate(
        [jnp.ones((b, 1), z_.dtype), z_, jnp.zeros((b, AUX), z_.dtype)], axis=1
    )
    iif = jnp.concatenate(
        [jnp.ones((b, 1), ii_.dtype), ii_, jnp.zeros((b, AUX), ii_.dtype)], axis=1
    )
    k1, k2 = jax.random.split(jax.random.key(42))
    v0 = jax.random.normal(k1, (b, KK), jnp.float32)
    v0 = v0 / jnp.linalg.norm(v0, axis=-1, keepdims=True)
    R = jax.random.normal(k2, (b, NV, KK), jnp.float32)
    R = R - jnp.einsum("bnk,bk->bn", R, v0)[..., None] * v0[:, None, :]
    R = R / jnp.linalg.norm(R, axis=-1, keepdims=True)
    c = jnp.cos(jnp.pi * z_full)[..., None]
    s = jnp.sin(jnp.pi * z_full)[..., None]
    V = jnp.where(iif[..., None] > 0, -c * v0[:, None, :] + s * R, R)
    V = V.at[:, 0].set(v0)
    return (
        np.asarray(V),
        np.asarray(v0),
        np.asarray(z_full),
        np.asarray(iif == 0),
    )


TRACE = False
LAST_RESULTS = None


def kernel(C, z, is_input):
    global LAST_RESULTS
    from concourse.bass_utils import run_bass_kernel_spmd

    C = np.ascontiguousarray(np.asarray(C, np.float32))
    z = np.asarray(z, np.float32)
    is_input = np.asarray(is_input)

    V0, v0, z_full, free = _host_init(z, is_input)

    CN4 = np.ascontiguousarray(-4.0 * C)
    U4 = np.stack(
        [np.triu(CN4[b * P : (b + 1) * P, b * P : (b + 1) * P], 1) for b in range(NB)]
    ).astype(np.float32)
    U4N = np.ascontiguousarray(-U4)
    IDN = np.eye(P, dtype=np.float32)

    in_maps = []
    pre = []
    for c in range(NCORES):
        bc = slice(c * BL, (c + 1) * BL)
        V0c = V0[bc]            # (BL, NV, KK)
        freec = free[bc]        # (BL, NV)
        freeT = freec.T         # (NV, BL)
        Vt = np.ascontiguousarray(V0c.transpose(1, 0, 2))  # (NV, BL, KK)
        vh0 = np.where(freeT[:, :, None], Vt, 0.0).reshape(NV, NBK)
        vfz = np.where(~freeT[:, :, None], Vt, 0.0).reshape(NV, NBK)
        mskc = (
            (BIGMASK * (1.0 - freeT.astype(np.float32)))
            .reshape(NB, P, BL)
            .transpose(1, 0, 2)
        )
        in_maps.append(
            dict(
                cn4=CN4,
                u4=U4,
                u4n=U4N,
                vh0=np.ascontiguousarray(vh0.astype(np.float32)),
                vfz=np.ascontiguousarray(vfz.astype(np.float32)),
                msk=np.ascontiguousarray(mskc.astype(np.float32)),
                idn=IDN,
            )
        )
        pre.append((V0c, freec))

    if "nc" not in _CACHE:
        _CACHE["nc"] = _build()
    nc = _CACHE["nc"]

    res = run_bass_kernel_spmd(
        nc, in_maps, core_ids=list(range(NCORES)), trace=TRACE
    )
    LAST_RESULTS = res

    Vfull = np.zeros((B, NV, KK), np.float32)
    for c in range(NCORES):
        V0c, freec = pre[c]
        vo = res.results[c]["vout"]  # (NV, NBK)
        Vdev = vo.reshape(NV, BL, KK).transpose(1, 0, 2)
        Vfull[c * BL : (c + 1) * BL] = np.where(freec[:, :, None], Vdev, V0c)

    cosang = np.clip(
        -np.einsum("bnk,bk->bn", Vfull, v0), -1.0 + 1e-7, 1.0 - 1e-7
    )
    zo = np.where(free, np.arccos(cosang) / np.pi, z_full)
    return np.ascontiguousarray(zo[:, 1 : NV - AUX].astype(np.float32))
